# revision 1
# baseline (speedup 1.0000x reference)
"""Causal self-attention (B=2, S=2048, E=1024, H=16, D=64) on 8 TRN2 cores.

Sharding: core c = (batch b = c//4, head-group g = c%4) owns batch b and
heads 4g..4g+3 (a 256-wide slice of the QKV projections / Wo rows).
Each core computes its partial out-projection y_c = attout_c @ Wo_c; the
host sums the 4 partials per batch and adds bo (the tensor-parallel
out-proj all-reduce, done on host since cores are independent).

All device inputs/outputs are host-packed into [128, X] layouts whose
per-partition rows are contiguous in DRAM, so every DMA is 128 large
contiguous descriptors (DMA-issue cost on the sync sequencer would
otherwise dominate the kernel).

Device pipeline (per core), bf16 operands / fp32 PSUM accumulation:
  Q^T, K^T [256, S] via lhsT=W chunk, rhs=xT chunk
  V natural [S, 4*(64+1)] with a ones column per head (softmax denom)
  scores^T [k, q] per head: lhsT=K^T (D=64 contraction, head pairs
  packed in PE row-groups 0-63/64-127), exp on ACT (scale=1/8 folded),
  causal mask multiply on DVE (diagonal blocks only; upper blocks
  skipped entirely)
  attout^T [65, q] PV matmul, row 64 = softmax denominator
  normalize: reciprocal_approx_fast + gpsimd partition_broadcast + TT mul
  y = attoutT_norm.T @ Wo_c, staged in SBUF, DMA'd out in 4 chunks.
"""

import numpy as np

B, S, E, H = 2, 2048, 1024, 16
D = E // H          # 64
NCORES = 8
HPC = 4             # heads per core
HD = HPC * D        # 256 cols per core
KT = E // 128       # 8 contraction tiles for projections
QC = S // 512       # 4 query chunks
NQT = S // 128      # 16 row tiles
VW = HPC * (D + 1)  # 260: V + ones column per head

_prog = None
LAST_RESULTS = None


def _build_program():
    import concourse.mybir as mybir
    import concourse.tile as tile
    from concourse import bacc, library_config

    f32 = mybir.dt.float32
    bf16 = mybir.dt.bfloat16
    Exp = mybir.ActivationFunctionType.Exp
    Identity = mybir.ActivationFunctionType.Identity

    nc = bacc.Bacc(trn_type="TRN2", target_bir_lowering=False, debug=False)

    xT = nc.dram_tensor("xT", [128, QC * KT * 512], bf16, kind="ExternalInput").ap()
    wq = nc.dram_tensor("wq", [128, KT * HD], bf16, kind="ExternalInput").ap()
    wk = nc.dram_tensor("wk", [128, KT * HD], bf16, kind="ExternalInput").ap()
    wv = nc.dram_tensor("wv", [128, KT * HD], bf16, kind="ExternalInput").ap()
    wo = nc.dram_tensor("wo", [128, 2 * E], bf16, kind="ExternalInput").ap()
    bq = nc.dram_tensor("bqc", [128, 2], f32, kind="ExternalInput").ap()
    bk = nc.dram_tensor("bkc", [128, 2], f32, kind="ExternalInput").ap()
    bv = nc.dram_tensor("bvb", [128, HD], bf16, kind="ExternalInput").ap()
    mask = nc.dram_tensor("mask", [128, 4 * 512], bf16, kind="ExternalInput").ap()
    y = nc.dram_tensor("y", [128, NQT * E], f32, kind="ExternalOutput").ap()

    with tile.TileContext(nc) as tc:
        with (
            tc.tile_pool(name="consts", bufs=1) as consts,
            tc.tile_pool(name="exps", bufs=4) as exps,
            tc.tile_pool(name="small", bufs=4) as small,
            tc.tile_pool(name="ps_sc", bufs=3, space="PSUM") as ps_sc,
            tc.tile_pool(name="ps_acc", bufs=2, space="PSUM") as ps_acc,
        ):
            nc.gpsimd.load_library(library_config.attn)
            # ---- constants; DMA order tuned so qc=0 work starts ASAP ----
            xt_sb = consts.tile([128, QC, KT, 512], bf16)
            wq_sb = consts.tile([128, KT, HD], bf16)
            wk_sb = consts.tile([128, KT, HD], bf16)
            wv_sb = consts.tile([128, KT, HD], bf16)
            wo_sb = consts.tile([128, 2, E], bf16)
            mask_sb = consts.tile([128, 4, 512], bf16)
            bq_sb = consts.tile([128, 2], f32)
            bk_sb = consts.tile([128, 2], f32)
            bv_sb = consts.tile([128, HD], bf16)

            def load_xt(qc):
                nc.sync.dma_start(
                    out=xt_sb[:, qc],
                    in_=xT[:, qc * KT * 512 : (qc + 1) * KT * 512].rearrange(
                        "p (kt c) -> p kt c", kt=KT
                    ),
                )

            nc.sync.dma_start(out=wq_sb, in_=wq.rearrange("p (kt c) -> p kt c", kt=KT))
            load_xt(0)
            nc.sync.dma_start(out=wk_sb, in_=wk.rearrange("p (kt c) -> p kt c", kt=KT))
            nc.sync.dma_start(out=wv_sb, in_=wv.rearrange("p (kt c) -> p kt c", kt=KT))
            nc.sync.dma_start(out=bq_sb, in_=bq)
            nc.sync.dma_start(out=bk_sb, in_=bk)
            nc.sync.dma_start(out=bv_sb, in_=bv)
            load_xt(1)
            nc.sync.dma_start(out=mask_sb, in_=mask.rearrange("p (t c) -> p t c", t=4))
            load_xt(2)
            load_xt(3)
            nc.sync.dma_start(out=wo_sb, in_=wo.rearrange("p (kt c) -> p kt c", kt=2))

            # ---- persistent activations ----
            # Q^T/K^T: [128, mt, S]; mt=0 holds cols 0-127 (heads 0,1),
            # mt=1 holds cols 128-255 (heads 2,3).
            qt_sb = consts.tile([128, 2, S], bf16)
            kt_sb = consts.tile([128, 2, S], bf16)
            # V natural: [row-in-tile, rt, 4*(64+1)]; per head h cols
            # h*65..h*65+63 are V, col h*65+64 is ones.
            v_sb = consts.tile([128, NQT, VW], bf16)
            nc.vector.memset(
                v_sb.rearrange("p rt (h c) -> p rt h c", h=HPC)[:, :, :, D : D + 1],
                1.0,
            )
            # normalized attout^T, same layout as qt_sb
            at_sb = consts.tile([128, 2, S], bf16)
            # full output staging: [p, qt, col]
            y_sb = consts.tile([128, NQT, E], f32)

            # ====== fused per-qc loop: projections -> attention -> out ======
            for qc in range(QC):
                # ---- projections for this q-chunk ----
                for w_sb, b_sb, dst in ((wq_sb, bq_sb, qt_sb), (wk_sb, bk_sb, kt_sb)):
                    ps = ps_sc.tile([128, 1024], f32, tag="sc", name=f"ps_qk{qc}")
                    for mt in range(2):
                        o = ps[:, mt * 512 : mt * 512 + 512]
                        for kt in range(KT):
                            nc.tensor.matmul(
                                o,
                                lhsT=w_sb[:, kt, mt * 128 : mt * 128 + 128],
                                rhs=xt_sb[:, qc, kt],
                                start=(kt == 0),
                                stop=(kt == KT - 1),
                            )
                        # PSUM->SBUF copy on DVE with the bias folded in
                        nc.vector.tensor_scalar_add(
                            dst[:, mt, qc * 512 : (qc + 1) * 512],
                            o,
                            b_sb[:, mt : mt + 1],
                        )

                for half in range(2):  # two V psum tiles, 2 row-tiles each
                    ps = ps_sc.tile([128, 1024], f32, tag="sc", name=f"ps_v{qc}_{half}")
                    for j in range(2):
                        rl = half * 2 + j          # row-tile within chunk (0..3)
                        rt = qc * 4 + rl           # global row tile
                        o = ps[:, j * 512 : j * 512 + HD]
                        for kt in range(KT):
                            nc.tensor.matmul(
                                o,
                                lhsT=xt_sb[:, qc, kt, rl * 128 : rl * 128 + 128],
                                rhs=wv_sb[:, kt, :],
                                start=(kt == 0),
                                stop=(kt == KT - 1),
                            )
                        # PSUM->SBUF with bias added (bvb host-broadcast)
                        nc.vector.tensor_add(
                            v_sb[:, rt, :]
                            .rearrange("p (h c) -> p h c", h=HPC)[:, :, 0:D],
                            o.rearrange("p (h c) -> p h c", h=HPC),
                            bv_sb.rearrange("p (h c) -> p h c", h=HPC),
                        )

                # ---- attention for this q-chunk, both head pairs ----
                nkt = 4 * (qc + 1)       # causal: k-tiles 0..nkt-1
                for mt in range(2):      # head pair (2mt, 2mt+1)
                    acc = [
                        ps_acc.tile([128, 512], f32, tag="acc", name=f"acc{mt}{qc}{j}")
                        for j in range(2)
                    ]
                    for kt in range(nkt):
                        t = kt - 4 * qc
                        # diagonal blocks: columns q < 128*t are fully masked
                        # -> narrow QK/exp/mask/PV to the valid range. PV
                        # never touches the dead columns (other kt wrote
                        # them), so no memset is needed.
                        off = 128 * t if t > 0 else 0
                        w = 512 - off
                        ps = ps_sc.tile([128, 1024], f32, tag="sc", name=f"ps_s{kt}")
                        for j in range(2):   # head within pair
                            pb = j * 64
                            nc.tensor.matmul(
                                ps[:, j * 512 + off : j * 512 + 512],
                                lhsT=kt_sb[pb : pb + 64, mt, kt * 128 : kt * 128 + 128],
                                rhs=qt_sb[
                                    pb : pb + 64, mt,
                                    qc * 512 + off : qc * 512 + 512,
                                ],
                                start=True,
                                stop=True,
                            )
                        ex = exps.tile([128, 1024], bf16, tag="ex", name=f"ex{kt}")
                        # scores scale 1/sqrt(D) folded into exp
                        if off == 0:
                            nc.scalar.activation(ex, ps, Exp, scale=0.125)
                        else:
                            for j in range(2):
                                nc.scalar.activation(
                                    ex[:, j * 512 + off : j * 512 + 512],
                                    ps[:, j * 512 + off : j * 512 + 512],
                                    Exp,
                                    scale=0.125,
                                )
                        for j in range(2):
                            exj = ex[:, j * 512 + off : j * 512 + 512]
                            if t >= 0:  # diagonal block: causal mask
                                nc.vector.tensor_mul(
                                    exj, exj, mask_sb[:, t, off:512]
                                )
                            h = 2 * mt + j
                            nc.tensor.matmul(
                                acc[j][0:65, off:512],
                                lhsT=v_sb[:, kt, h * 65 : h * 65 + 65],
                                rhs=exj,
                                start=(kt == 0),
                                stop=(kt == nkt - 1),
                            )
                    for j in range(2):
                        dn = small.tile([1, 512], f32, tag="dn", name=f"dn{j}")
                        # reciprocal_approx_fast misreads PSUM on HW; bounce
                        # the denominator row through SBUF first.
                        nc.vector.tensor_copy(dn, acc[j][64:65, :])
                        rc = small.tile([1, 512], f32, tag="rc", name=f"rc{j}")
                        nc.vector.reciprocal_approx_fast(out=rc, in_=dn)
                        bc = small.tile([64, 512], f32, tag="bc", name=f"bc{j}")
                        nc.gpsimd.partition_broadcast(out_ap=bc, in_ap=rc)
                        pb = j * 64
                        nc.vector.tensor_mul(
                            at_sb[pb : pb + 64, mt, qc * 512 : qc * 512 + 512],
                            acc[j][0:64, :],
                            bc,
                        )

                # ---- out projection for this quarter ----
                for qt in range(qc * 4, qc * 4 + 4):
                    for nh in range(2):
                        ps = ps_acc.tile(
                            [128, 512], f32, tag="acc", name=f"ps_y{qt}{nh}"
                        )
                        for kt2 in range(2):
                            nc.tensor.matmul(
                                ps,
                                lhsT=at_sb[:, kt2, qt * 128 : qt * 128 + 128],
                                rhs=wo_sb[:, kt2, nh * 512 : nh * 512 + 512],
                                start=(kt2 == 0),
                                stop=(kt2 == 1),
                            )
                        nc.vector.tensor_copy(
                            y_sb[:, qt, nh * 512 : nh * 512 + 512], ps
                        )
                nc.sync.dma_start(
                    out=y[:, qc * 4 * E : (qc + 1) * 4 * E],
                    in_=y_sb[:, qc * 4 : (qc + 1) * 4, :],
                )

    nc.compile()
    return nc


def _get_program():
    global _prog
    if _prog is None:
        _prog = _build_program()
    return _prog


def _make_mask():
    import ml_dtypes

    k = np.arange(128)[:, None]
    q = np.arange(512)[None, :]
    m = np.stack([(q >= k + 128 * t) for t in range(4)])  # [4, 128, 512]
    return np.ascontiguousarray(
        m.transpose(1, 0, 2).reshape(128, 4 * 512)
    ).astype(ml_dtypes.bfloat16)


def _pack_rows(a, ktiles):
    """[ktiles*128, C] -> [128, ktiles*C] with per-partition contiguous rows."""
    kt, c = ktiles, a.shape[1]
    return np.ascontiguousarray(
        a.reshape(kt, 128, c).transpose(1, 0, 2).reshape(128, kt * c)
    )


def _core_inputs(x, Wq, bq, Wk, bk, Wv, bv, Wo, mask, c):
    import ml_dtypes

    bf16 = ml_dtypes.bfloat16
    b, g = divmod(c, 4)
    sl = slice(g * HD, (g + 1) * HD)
    xT = x[b].T  # [E, S]
    xT_p = np.ascontiguousarray(
        xT.reshape(KT, 128, QC, 512).transpose(1, 2, 0, 3).reshape(128, QC * KT * 512)
    )
    return {
        "xT": xT_p.astype(bf16),
        "wq": _pack_rows(Wq[:, sl], KT).astype(bf16),
        "wk": _pack_rows(Wk[:, sl], KT).astype(bf16),
        "wv": _pack_rows(Wv[:, sl], KT).astype(bf16),
        "wo": _pack_rows(Wo[sl, :], 2).astype(bf16),
        "bqc": np.ascontiguousarray(bq[sl].reshape(2, 128).T).astype(np.float32),
        "bkc": np.ascontiguousarray(bk[sl].reshape(2, 128).T).astype(np.float32),
        "bvb": np.ascontiguousarray(
            np.broadcast_to(bv[sl], (128, HD))
        ).astype(bf16),
        "mask": mask,
    }


def _unpack_y(y_p):
    """[128, NQT*E] -> [S, E]"""
    return y_p.reshape(128, NQT, E).transpose(1, 0, 2).reshape(S, E)


def kernel(x, Wq, bq, Wk, bk, Wv, bv, Wo, bo, **_run_kwargs):
    from concourse.bass_utils import run_bass_kernel_spmd

    x = np.asarray(x, dtype=np.float32)
    Wq, bq = np.asarray(Wq, np.float32), np.asarray(bq, np.float32)
    Wk, bk = np.asarray(Wk, np.float32), np.asarray(bk, np.float32)
    Wv, bv = np.asarray(Wv, np.float32), np.asarray(bv, np.float32)
    Wo, bo = np.asarray(Wo, np.float32), np.asarray(bo, np.float32)

    nc = _get_program()
    mask = _make_mask()
    in_maps = [
        _core_inputs(x, Wq, bq, Wk, bk, Wv, bv, Wo, mask, c) for c in range(NCORES)
    ]
    res = run_bass_kernel_spmd(nc, in_maps, list(range(NCORES)), **_run_kwargs)
    global LAST_RESULTS
    LAST_RESULTS = res
    parts = [_unpack_y(res.results[c]["y"]) for c in range(NCORES)]
    out = np.empty((B, S, E), np.float32)
    for b in range(B):
        out[b] = parts[4 * b] + parts[4 * b + 1] + parts[4 * b + 2] + parts[4 * b + 3]
        out[b] += bo
    return out



# revision 29
# speedup vs baseline: 1.2217x; 1.2217x over previous
"""Causal self-attention (B=2, S=2048, E=1024, H=16, D=64) on 8 TRN2 cores.

Sharding: core c = (batch b = c//4, head-group g = c%4) owns batch b and
heads 4g..4g+3 (a 256-wide slice of the QKV projections / Wo rows).
Each core computes its partial out-projection y_c = attout_c @ Wo_c; the
host sums the 4 partials per batch and adds the folded bias (the
tensor-parallel out-proj all-reduce, done on host since cores are
independent).

Bias identities (exact): the K bias drops out of softmax entirely
(per-query constant shift); the V bias contributes bv @ Wo to y since
softmax weights sum to 1 — both folded into the host-side bias add, so
the device only applies bq.

All device inputs/outputs are host-packed into [128, X] layouts whose
per-partition rows are contiguous in DRAM, so every DMA is 128 large
contiguous descriptors.

Device pipeline (per core), bf16 operands / fp32 PSUM accumulation:
  Q^T, K^T [256, S] via lhsT=W chunk, rhs=xT chunk
  V natural [S, 4*(64+1)] with a ones column per head (softmax denom)
  scores^T [k, q] per head: lhsT=K^T (D=64 contraction, head pairs
  packed in PE row-groups 0-63/64-127), exp on ACT (scale=1/8 folded,
  one j-strided call per k-tile), causal mask multiply on GPSIMD
  (diagonal blocks only), attout^T [65, q] PV matmul with row 64 the
  softmax denominator, normalize via reciprocal + partition_broadcast.

The attention phase is ACT(exp)-bound, so projection and out-projection
matmuls for neighboring chunks are interleaved into the PE issue stream
as filler between score tiles: engine queues are in-order, so this is
the only way PE can make progress while a scores PSUM buffer waits on
its exp.  Schedule: proj(0) prologue; attention(qc) carries
proj(qc+1) + out-proj(qc-1) as filler; out-proj(3) epilogue.
"""

import numpy as np

B, S, E, H = 2, 2048, 1024, 16
D = E // H          # 64
NCORES = 8
HPC = 4             # heads per core
HD = HPC * D        # 256 cols per core
KT = E // 128       # 8 contraction tiles for projections
QC = S // 512       # 4 query chunks
NQT = S // 128      # 16 row tiles
VW = HPC * (D + 1)  # 260: V + ones column per head

_prog = None
LAST_RESULTS = None


def _build_program():
    import concourse.mybir as mybir
    import concourse.tile as tile
    from concourse import bacc, library_config

    f32 = mybir.dt.float32
    bf16 = mybir.dt.bfloat16
    Exp = mybir.ActivationFunctionType.Exp

    nc = bacc.Bacc(trn_type="TRN2", target_bir_lowering=False, debug=False)

    xT = nc.dram_tensor("xT", [128, QC * KT * 512], bf16, kind="ExternalInput").ap()
    wq = nc.dram_tensor("wq", [128, KT * HD], bf16, kind="ExternalInput").ap()
    wk = nc.dram_tensor("wk", [128, KT * HD], bf16, kind="ExternalInput").ap()
    wv = nc.dram_tensor("wv", [128, KT * HD], bf16, kind="ExternalInput").ap()
    wo = nc.dram_tensor("wo", [128, 2 * E], bf16, kind="ExternalInput").ap()
    bq = nc.dram_tensor("bqc", [128, 2], f32, kind="ExternalInput").ap()
    # single lower-triangular [128,128] band mask (valid iff q_local >= k):
    # within a diagonal block only the leading 128-wide column band mixes
    # valid/invalid entries; every column beyond it is fully valid, and the
    # band pattern is the same for every diagonal tile.
    mask = nc.dram_tensor("mask", [128, 128], bf16, kind="ExternalInput").ap()
    y = nc.dram_tensor("y", [128, NQT * E], bf16, kind="ExternalOutput").ap()

    with tile.TileContext(nc) as tc:
        with (
            tc.tile_pool(name="consts", bufs=1) as consts,
            tc.tile_pool(name="exps", bufs=5) as exps,
            tc.tile_pool(name="small", bufs=4) as small,
            tc.tile_pool(name="ps_sc", bufs=2, space="PSUM") as ps_sc,
            tc.tile_pool(name="ps_pj", bufs=2, space="PSUM") as ps_pj,
            tc.tile_pool(name="ps_acc", bufs=2, space="PSUM") as ps_acc,
        ):
            # ---- constants; DMA order tuned so qc=0 work starts ASAP ----
            xt_sb = consts.tile([128, QC, KT, 512], bf16)
            wq_sb = consts.tile([128, KT, HD], bf16)
            wk_sb = consts.tile([128, KT, HD], bf16)
            wv_sb = consts.tile([128, KT, HD], bf16)
            wo_sb = consts.tile([128, 2, E], bf16)
            mask_sb = consts.tile([128, 128], bf16)
            bq_sb = consts.tile([128, 2], f32)

            # the DMA pipe services transfers roughly in issue order, so
            # issue strictly in need order; quarter-granular xt(0) pieces
            # get the first projection matmuls started sooner
            def load_xt(qc, quarter=None):
                ks = (
                    slice(0, KT)
                    if quarter is None
                    else slice(quarter * 2, quarter * 2 + 2)
                )
                nkt = ks.stop - ks.start
                nc.sync.dma_start(
                    out=xt_sb[:, qc, ks],
                    in_=xT[
                        :, qc * KT * 512 + ks.start * 512 : qc * KT * 512 + ks.stop * 512
                    ].rearrange("p (kt c) -> p kt c", kt=nkt),
                )

            def load_w(dst, src, half):
                ks = slice(half * 4, half * 4 + 4)
                nc.sync.dma_start(
                    out=dst[:, ks],
                    in_=src[:, ks.start * HD : ks.stop * HD].rearrange(
                        "p (kt c) -> p kt c", kt=4
                    ),
                )

            load_w(wq_sb, wq, 0)
            load_xt(0, 0)
            load_xt(0, 1)
            load_w(wq_sb, wq, 1)
            load_xt(0, 2)
            load_xt(0, 3)
            load_w(wk_sb, wk, 0)
            load_w(wk_sb, wk, 1)
            nc.sync.dma_start(out=wv_sb, in_=wv.rearrange("p (kt c) -> p kt c", kt=KT))
            nc.sync.dma_start(out=bq_sb, in_=bq)
            nc.sync.dma_start(out=mask_sb, in_=mask)
            # library load is only needed by normalize (partition_broadcast),
            # ~15us in — keep it off the startup DMA path
            nc.gpsimd.load_library(library_config.attn)
            load_xt(1)
            nc.sync.dma_start(out=wo_sb, in_=wo.rearrange("p (kt c) -> p kt c", kt=2))
            load_xt(2)
            load_xt(3)

            # force the Exp activation-table load at t~0, off the first
            # real exp's critical path
            warm = small.tile([1, 8], f32, tag="warm", name="warm")
            nc.vector.memset(warm, 0.0)
            warm2 = small.tile([1, 8], f32, tag="warm", name="warm2")
            nc.scalar.activation(warm2, warm, Exp)

            # ---- persistent activations ----
            # Q^T/K^T: [128, mt, S]; mt=0 holds cols 0-127 (heads 0,1),
            # mt=1 holds cols 128-255 (heads 2,3).
            qt_sb = consts.tile([128, 2, S], bf16)
            kt_sb = consts.tile([128, 2, S], bf16)
            # V natural: [row-in-tile, rt, 4*(64+1)]; per head h cols
            # h*65..h*65+63 are V, col h*65+64 is ones.
            v_sb = consts.tile([128, NQT, VW], bf16)
            nc.vector.memset(
                v_sb.rearrange("p rt (h c) -> p rt h c", h=HPC)[:, :, :, D : D + 1],
                1.0,
            )
            # normalized attout^T, same layout as qt_sb
            at_sb = consts.tile([128, 2, S], bf16)
            # output staging (bf16 partials; host accumulates in fp32)
            y_sb = consts.tile([128, NQT, E], bf16)

            # ---- filler units: projections and out-projections ----
            # Each closure emits ~0.9us of PE work (4 matmuls); they are
            # issued between attention score tiles so the PE stays busy
            # while a scores PSUM buffer waits on its exp.
            def proj_fillers(qc):
                fs = []
                for w_sb, kind in ((wq_sb, "q"), (wk_sb, "k")):
                    for mt in range(2):
                        box = {}

                        def h1(qc=qc, w_sb=w_sb, mt=mt, kind=kind, box=box):
                            ps = ps_pj.tile(
                                [128, 512], f32, tag="pj", name=f"pj_{kind}{qc}{mt}"
                            )
                            box["ps"] = ps
                            for kt in range(4):
                                nc.tensor.matmul(
                                    ps,
                                    lhsT=w_sb[:, kt, mt * 128 : mt * 128 + 128],
                                    rhs=xt_sb[:, qc, kt],
                                    start=(kt == 0),
                                    stop=False,
                                )

                        def h2(qc=qc, w_sb=w_sb, mt=mt, kind=kind, box=box):
                            ps = box["ps"]
                            for kt in range(4, 8):
                                nc.tensor.matmul(
                                    ps,
                                    lhsT=w_sb[:, kt, mt * 128 : mt * 128 + 128],
                                    rhs=xt_sb[:, qc, kt],
                                    start=False,
                                    stop=(kt == 7),
                                )
                            dst = qt_sb if kind == "q" else kt_sb
                            if kind == "q":
                                # PSUM->SBUF copy on DVE with the bias folded in
                                nc.vector.tensor_scalar_add(
                                    dst[:, mt, qc * 512 : (qc + 1) * 512],
                                    ps,
                                    bq_sb[:, mt : mt + 1],
                                )
                            else:
                                nc.vector.tensor_copy(
                                    dst[:, mt, qc * 512 : (qc + 1) * 512], ps
                                )

                        fs += [h1, h2]
                for rl in range(4):
                    rt = qc * 4 + rl
                    box = {}

                    def v1(qc=qc, rl=rl, box=box):
                        ps = ps_pj.tile([128, 512], f32, tag="pj", name=f"pj_v{qc}{rl}")
                        box["ps"] = ps
                        for kt in range(4):
                            nc.tensor.matmul(
                                ps[:, 0:HD],
                                lhsT=xt_sb[:, qc, kt, rl * 128 : rl * 128 + 128],
                                rhs=wv_sb[:, kt, :],
                                start=(kt == 0),
                                stop=False,
                            )

                    def v2(rt=rt, qc=qc, rl=rl, box=box):
                        ps = box["ps"]
                        for kt in range(4, 8):
                            nc.tensor.matmul(
                                ps[:, 0:HD],
                                lhsT=xt_sb[:, qc, kt, rl * 128 : rl * 128 + 128],
                                rhs=wv_sb[:, kt, :],
                                start=False,
                                stop=(kt == 7),
                            )
                        nc.vector.tensor_copy(
                            v_sb[:, rt, :].rearrange("p (h c) -> p h c", h=HPC)[
                                :, :, 0:D
                            ],
                            ps[:, 0:HD].rearrange("p (h c) -> p h c", h=HPC),
                        )

                    fs += [v1, v2]
                return fs

            def outproj_fillers(qc, use_act=False):
                # use_act: alternate PSUM->SBUF copies between DVE and ACT —
                # only safe when ACT has no exp work left (the epilogue)
                fs = []
                for qt in range(qc * 4, qc * 4 + 4):
                    for nh in range(2):

                        def f(qt=qt, nh=nh):
                            ps = ps_pj.tile(
                                [128, 512], f32, tag="pj", name=f"pj_y{qt}{nh}"
                            )
                            for kt2 in range(2):
                                nc.tensor.matmul(
                                    ps,
                                    lhsT=at_sb[:, kt2, qt * 128 : qt * 128 + 128],
                                    rhs=wo_sb[:, kt2, nh * 512 : nh * 512 + 512],
                                    start=(kt2 == 0),
                                    stop=(kt2 == 1),
                                )
                            dst = y_sb[:, qt, nh * 512 : nh * 512 + 512]
                            if use_act and nh == 0:
                                nc.scalar.copy(dst, ps)
                            else:
                                nc.vector.tensor_copy(dst, ps)
                            if nh == 1:
                                nc.sync.dma_start(
                                    out=y[:, qt * E : (qt + 1) * E],
                                    in_=y_sb[:, qt, :],
                                )

                        fs.append(f)
                return fs

            # ---- attention for one q-chunk, fillers interleaved ----
            def attention(qc, fillers, pre_tail=None):
                nkt = 4 * (qc + 1)       # causal: k-tiles 0..nkt-1
                ntiles = 2 * nkt
                ti = fi = 0
                for mt in range(2):      # head pair (2mt, 2mt+1)
                    acc = [
                        ps_acc.tile([128, 512], f32, tag="acc", name=f"acc{mt}{qc}{j}")
                        for j in range(2)
                    ]

                    def pv(kt, ex, off):
                        for j in range(2):
                            h = 2 * mt + j
                            nc.tensor.matmul(
                                acc[j][0:65, off:512],
                                lhsT=v_sb[:, kt, h * 65 : h * 65 + 65],
                                rhs=ex[:, j, off:512],
                                start=(kt == 0),
                                stop=(kt == nkt - 1),
                            )

                    pend = []  # PV pipelined two tiles behind the exp/mask
                    for kt in range(nkt):
                        t = kt - 4 * qc
                        # diagonal blocks: columns q < 128*t are fully masked
                        # -> narrow QK/exp/mask/PV to the valid range. PV
                        # never touches the dead columns (other kt wrote
                        # them), so no memset is needed.
                        off = 128 * t if t > 0 else 0
                        ps = ps_sc.tile(
                            [128, 2, 512], f32, tag="sc", name=f"sc{qc}{mt}{kt}"
                        )
                        for j in range(2):   # head within pair
                            pb = j * 64
                            nc.tensor.matmul(
                                ps[:, j, off:512],
                                lhsT=kt_sb[pb : pb + 64, mt, kt * 128 : kt * 128 + 128],
                                rhs=qt_sb[
                                    pb : pb + 64, mt,
                                    qc * 512 + off : qc * 512 + 512,
                                ],
                                start=True,
                                stop=True,
                            )
                        ex = exps.tile([128, 2, 512], bf16, tag="ex", name=f"ex{kt}")
                        # scores scale 1/sqrt(D) folded into exp; one
                        # j-strided call per k-tile
                        nc.scalar.activation(
                            ex[:, :, off:512], ps[:, :, off:512], Exp, scale=0.125
                        )
                        if t >= 0:
                            # diagonal block: only the leading 128-wide band
                            # mixes valid/masked; multiply it by the shared
                            # lower-tri mask (cheap DVE op, off the PV chain)
                            for j in range(2):
                                nc.vector.tensor_mul(
                                    ex[:, j, off : off + 128],
                                    ex[:, j, off : off + 128],
                                    mask_sb,
                                )
                        ti += 1
                        want = ti * len(fillers) // ntiles
                        while fi < want:
                            fillers[fi]()
                            fi += 1
                        if len(pend) == 2:
                            pv(*pend.pop(0))
                        pend.append((kt, ex, off))
                    for p in pend:
                        pv(*p)
                    if mt == 1 and pre_tail is not None:
                        pre_tail()
                    # normalize, one chain per head; the j=1 PSUM->SBUF
                    # bounce goes to ACT only when ACT has no exp work left.
                    # (reciprocal_approx_fast misreads PSUM on HW, hence the
                    # SBUF bounce.)
                    for j in range(2):
                        dn = small.tile([1, 512], f32, tag="dn", name=f"dn{j}")
                        if j == 1 and qc == QC - 1 and mt == 1:
                            nc.scalar.copy(dn, acc[j][64:65, :])
                        else:
                            nc.vector.tensor_copy(dn, acc[j][64:65, :])
                        rc = small.tile([1, 512], f32, tag="rc", name=f"rc{j}")
                        nc.vector.reciprocal_approx_fast(out=rc, in_=dn)
                        bc = small.tile([64, 512], f32, tag="bc", name=f"bc{j}")
                        nc.gpsimd.partition_broadcast(out_ap=bc, in_ap=rc)
                        pb = j * 64
                        nc.vector.tensor_mul(
                            at_sb[pb : pb + 64, mt, qc * 512 : qc * 512 + 512],
                            acc[j][0:64, :],
                            bc,
                        )
                while fi < len(fillers):
                    fillers[fi]()
                    fi += 1

            # ---- schedule ----
            # qc=3's attention is exp(ACT)-bound, so it gets the
            # ACT-independent out-projections of chunks 0-2 as filler;
            # earlier chunks carry the next chunk's projections.
            for f in proj_fillers(0):
                f()
            attention(0, proj_fillers(1))
            attention(1, proj_fillers(2))
            attention(2, proj_fillers(3))

            # split epilogue: the mt=0 halves of out-proj(3) only need the
            # already-normalized at_sb mt=0, so they are issued right after
            # the last PV drain (pre_tail) to keep PE busy and warm through
            # the final normalize chain; the mt=1 halves + copies follow.
            ep_groups = [(qt, nh) for qt in range(12, 16) for nh in range(2)]
            ep_slots = {}

            def ep_phase_a():
                slots = [
                    ps_pj.tile([128, 512], f32, tag="pj", name="ep_pj0"),
                    ps_pj.tile([128, 512], f32, tag="pj", name="ep_pj1"),
                ]
                for i in range(2):
                    sc = ps_sc.tile([128, 2, 512], f32, tag="sc", name=f"ep_sc{i}")
                    slots += [sc[:, 0, :], sc[:, 1, :]]
                for i, (qt, nh) in enumerate(ep_groups[:6]):
                    ep_slots[(qt, nh)] = slots[i]
                    nc.tensor.matmul(
                        slots[i],
                        lhsT=at_sb[:, 0, qt * 128 : qt * 128 + 128],
                        rhs=wo_sb[:, 0, nh * 512 : nh * 512 + 512],
                        start=True,
                        stop=False,
                    )

            attention(
                3,
                outproj_fillers(0) + outproj_fillers(1) + outproj_fillers(2),
                pre_tail=ep_phase_a,
            )

            for i, (qt, nh) in enumerate(ep_groups):
                if (qt, nh) in ep_slots:
                    ps = ep_slots[(qt, nh)]
                    nc.tensor.matmul(
                        ps,
                        lhsT=at_sb[:, 1, qt * 128 : qt * 128 + 128],
                        rhs=wo_sb[:, 1, nh * 512 : nh * 512 + 512],
                        start=False,
                        stop=True,
                    )
                else:
                    ps = ps_pj.tile([128, 512], f32, tag="pj", name=f"ep_y{qt}{nh}")
                    for kt2 in range(2):
                        nc.tensor.matmul(
                            ps,
                            lhsT=at_sb[:, kt2, qt * 128 : qt * 128 + 128],
                            rhs=wo_sb[:, kt2, nh * 512 : nh * 512 + 512],
                            start=(kt2 == 0),
                            stop=(kt2 == 1),
                        )
                dst = y_sb[:, qt, nh * 512 : nh * 512 + 512]
                if nh == 0:
                    nc.scalar.copy(dst, ps)
                else:
                    nc.vector.tensor_copy(dst, ps)
                    nc.sync.dma_start(
                        out=y[:, qt * E : (qt + 1) * E], in_=y_sb[:, qt, :]
                    )

    nc.compile()
    return nc


def _get_program():
    global _prog
    if _prog is None:
        _prog = _build_program()
    return _prog


def _make_mask():
    import ml_dtypes

    k = np.arange(128)[:, None]
    q = np.arange(128)[None, :]
    return np.ascontiguousarray(q >= k).astype(ml_dtypes.bfloat16)


def _pack_rows(a, ktiles):
    """[ktiles*128, C] -> [128, ktiles*C] with per-partition contiguous rows."""
    kt, c = ktiles, a.shape[1]
    return np.ascontiguousarray(
        a.reshape(kt, 128, c).transpose(1, 0, 2).reshape(128, kt * c)
    )


def _core_inputs(x, Wq, bq, Wk, Wv, Wo, mask, c):
    import ml_dtypes

    bf16 = ml_dtypes.bfloat16
    b, g = divmod(c, 4)
    sl = slice(g * HD, (g + 1) * HD)
    xT = x[b].T  # [E, S]
    xT_p = np.ascontiguousarray(
        xT.reshape(KT, 128, QC, 512).transpose(1, 2, 0, 3).reshape(128, QC * KT * 512)
    )
    return {
        "xT": xT_p.astype(bf16),
        "wq": _pack_rows(Wq[:, sl], KT).astype(bf16),
        "wk": _pack_rows(Wk[:, sl], KT).astype(bf16),
        "wv": _pack_rows(Wv[:, sl], KT).astype(bf16),
        "wo": _pack_rows(Wo[sl, :], 2).astype(bf16),
        "bqc": np.ascontiguousarray(bq[sl].reshape(2, 128).T).astype(np.float32),
        "mask": mask,
    }


def _unpack_y(y_p):
    """[128, NQT*E] -> [S, E]"""
    return y_p.reshape(128, NQT, E).transpose(1, 0, 2).reshape(S, E)


def kernel(x, Wq, bq, Wk, bk, Wv, bv, Wo, bo, **_run_kwargs):
    from concourse.bass_utils import run_bass_kernel_spmd

    x = np.asarray(x, dtype=np.float32)
    Wq, bq = np.asarray(Wq, np.float32), np.asarray(bq, np.float32)
    Wk, bk = np.asarray(Wk, np.float32), np.asarray(bk, np.float32)
    Wv, bv = np.asarray(Wv, np.float32), np.asarray(bv, np.float32)
    Wo, bo = np.asarray(Wo, np.float32), np.asarray(bo, np.float32)

    nc = _get_program()
    mask = _make_mask()
    in_maps = [
        _core_inputs(x, Wq, bq, Wk, Wv, Wo, mask, c) for c in range(NCORES)
    ]
    res = run_bass_kernel_spmd(nc, in_maps, list(range(NCORES)), **_run_kwargs)
    global LAST_RESULTS
    LAST_RESULTS = res
    parts = [_unpack_y(res.results[c]["y"].astype(np.float32)) for c in range(NCORES)]
    # bias identities: bk drops out of softmax; bv contributes bv @ Wo
    # (softmax weights sum to 1); both folded in here with bo.
    bias = bo + bv @ Wo
    out = np.empty((B, S, E), np.float32)
    for b in range(B):
        out[b] = parts[4 * b] + parts[4 * b + 1] + parts[4 * b + 2] + parts[4 * b + 3]
        out[b] += bias
    return out
